# revision 31
# baseline (speedup 1.0000x reference)
"""Trainium2 Bass kernel for nn_ActorTransformer (B=16, T=1024, D=96, L=6, NH=6).

Strategy: pure data parallelism, 2 batch elements per NeuronCore on 8 cores.
Inside a core, activations live transposed as [D=96 partitions, T=1024 free].
Attention is computed as attT [Tk, Tq] blocks so the AV matmul needs no
transposes; per-head matmuls are packed into 32-row/col PE tile groups and
run in bf16. Softmax denominators come from all-ones lhsT matmuls that land
replicated on 32-aligned PSUM rows. Weights are pre-cast/pre-laid-out on the
host and DMAed once.
"""

import os
import sys

import ml_dtypes
import numpy as np

sys.path.insert(0, "/opt/trn_rl_repo")

import concourse.bass as bass
import concourse.mybir as mybir
import concourse.tile as tile
from concourse import bacc
from concourse.bass_utils import run_bass_kernel_spmd
from concourse.masks import make_identity

# Model constants
L, NH, D, HS, FF = 6, 6, 96, 16, 384
DG, DO, DA = 32, 64, 16
T, B = 1024, 16
NCORES = 8
BPC = B // NCORES  # batch per core
SCALE = HS ** -0.5  # 0.25
EPS = 1e-5
CW = 512  # chunk width (matmul moving free dim)
NCH = T // CW  # 2
NT = T // 128  # 8 token tiles
F32 = mybir.dt.float32
BF16 = mybir.dt.bfloat16
NPBF = ml_dtypes.bfloat16
ATT_F32 = os.environ.get("ATT_F32", "0") == "1"
EDT = F32 if ATT_F32 else BF16

LAST_RESULT = None


def _dt(x):
    return np.ascontiguousarray(np.asarray(x, dtype=np.float32))


def build(nc):
    """Builds the full forward pass for BPC batch elements on one core."""
    def din(name, shape, dt=F32):
        return nc.dram_tensor(name, list(shape), dt, kind="ExternalInput").ap()

    goals = din("goals", [BPC, 1, DG])
    obss = din("obss", [BPC, T, DO])
    w_go = din("w_go", [DG + DO, D])
    b_go = din("b_go", [D])
    posT_d = din("posT", [D, T])
    wqa = din("wqa", [L, D, 112], BF16)
    wqb = din("wqb", [L, D, 48], BF16)
    wka = din("wka", [L, D, 112], BF16)
    wkb = din("wkb", [L, D, 48], BF16)
    wv_hd = din("wv_hd", [L, D, NH * HS], BF16)
    wpa = din("wpa", [L, 128, D], BF16)
    wpb = din("wpb", [L, 64, D], BF16)
    wf1 = din("wf1", [L, D, FF], BF16)
    wf2 = din("wf2", [L, FF, D], BF16)
    wact_d = din("wact", [D, DA], BF16)
    ln1_g = din("ln1_g", [L, D])
    ln1_b = din("ln1_b", [L, D])
    ln2_g = din("ln2_g", [L, D])
    ln2_b = din("ln2_b", [L, D])
    b_proj = din("b_proj", [L, D])
    b_ff1 = din("b_ff1", [L, FF])
    b_ff2 = din("b_ff2", [L, D])
    lnf_g = din("lnf_g", [D])
    lnf_b = din("lnf_b", [D])
    b_act = din("b_act", [DA])
    out = nc.dram_tensor("out", [BPC, T, DA], F32, kind="ExternalOutput").ap()
    if os.environ.get("KDEBUG"):
        nc._dbg = {
            "xemb": nc.dram_tensor("xemb", [D, T], F32, kind="ExternalOutput").ap(),
            "qa0": nc.dram_tensor("qa0", [112, T], F32, kind="ExternalOutput").ap(),
            "va0": nc.dram_tensor("va0", [128, 96 * NT], F32, kind="ExternalOutput").ap(),
            "oc0": nc.dram_tensor("oc0", [128, T], F32, kind="ExternalOutput").ap(),
            "xl0": nc.dram_tensor("xl0", [D, T], F32, kind="ExternalOutput").ap(),
        }

    with tile.TileContext(nc) as tc:
        _body(tc, goals, obss, w_go, b_go, posT_d, wqa, wqb, wka, wkb, wv_hd,
              wpa, wpb, wf1, wf2, wact_d, ln1_g, ln1_b, ln2_g, ln2_b,
              b_proj, b_ff1, b_ff2, lnf_g, lnf_b, b_act, out)
    return nc


def _body(tc, goals, obss, w_go, b_go, posT_d, wqa, wqb, wka, wkb, wv_hd,
          wpa, wpb, wf1, wf2, wact_d, ln1_g, ln1_b, ln2_g, ln2_b,
          b_proj, b_ff1, b_ff2, lnf_g, lnf_b, b_act, out):
    from contextlib import ExitStack
    nc = tc.nc
    ctx = ExitStack()

    wp = ctx.enter_context(tc.tile_pool(name="wp", bufs=1))     # weights/consts
    xp = ctx.enter_context(tc.tile_pool(name="xp", bufs=1))     # residual streams
    tp = ctx.enter_context(tc.tile_pool(name="tp", bufs=2))     # transient small
    xnp = ctx.enter_context(tc.tile_pool(name="xnp", bufs=2))   # LN outputs
    qkp = ctx.enter_context(tc.tile_pool(name="qkp", bufs=2))   # q/k spreads
    vp = ctx.enter_context(tc.tile_pool(name="vp", bufs=2))     # v tiles
    ep = ctx.enter_context(tc.tile_pool(name="ep", bufs=12))    # exp(att) blocks
    op_ = ctx.enter_context(tc.tile_pool(name="op", bufs=2))    # attn out concat
    hp = ctx.enter_context(tc.tile_pool(name="hp", bufs=1))     # ff hidden
    pp = ctx.enter_context(tc.tile_pool(name="pp", bufs=8, space="PSUM"))

    def psum(p, f, tag="ps"):
        return pp.tile([p, f], F32, tag=tag, name=tag)

    # ---------------- constants ----------------
    ident = wp.tile([128, 128], F32, tag="ident")
    make_identity(nc, ident[:])
    ones96 = wp.tile([96, 1], F32, tag="ones96")
    nc.vector.memset(ones96[:], 1.0)
    eps_c = wp.tile([1, 1], F32, tag="eps_c")
    nc.vector.memset(eps_c[:], EPS)
    ones128 = wp.tile([128, HS], EDT, tag="ones128")
    nc.vector.memset(ones128[:], 1.0)

    # b_act broadcast to [128, DA] (free-dim bias for the final head)
    bact_row = wp.tile([1, DA], F32, tag="bact_row")
    nc.sync.dma_start(out=bact_row[:], in_=b_act[None, :])
    bact_b = wp.tile([128, DA], F32, tag="bact_b")
    nc.gpsimd.partition_broadcast(bact_b[:], bact_row[:])

    posT = wp.tile([D, T], F32, tag="posT")
    nc.sync.dma_start(out=posT[:], in_=posT_d[:, :])

    wgo_g = wp.tile([DG, D], F32, tag="wgo_g")
    nc.sync.dma_start(out=wgo_g[:], in_=w_go[0:DG, :])
    wgo_o = wp.tile([DO, D], F32, tag="wgo_o")
    nc.sync.dma_start(out=wgo_o[:], in_=w_go[DG:DG + DO, :])
    bgo = wp.tile([D, 1], F32, tag="bgo")
    nc.sync.dma_start(out=bgo[:], in_=b_go[:, None])

    # per-layer weights: direct bf16 DMAs (host pre-laid-out)
    WQA, WQB, WKA, WKB, WV, WP, WF1, WF2 = [], [], [], [], [], [], [], []
    L1G, L1B, L2G, L2B, BP, BF1, BF2 = [], [], [], [], [], [], []

    def wtile(src, shape, tag, dt=BF16):
        t_ = wp.tile(list(shape), dt, tag=tag, name=tag)
        nc.sync.dma_start(out=t_[:], in_=src)
        return t_

    for l in range(L):
        WQA.append(wtile(wqa[l], [D, 112], f"wqA{l}"))
        WQB.append(wtile(wqb[l], [D, 48], f"wqB{l}"))
        WKA.append(wtile(wka[l], [D, 112], f"wkA{l}"))
        WKB.append(wtile(wkb[l], [D, 48], f"wkB{l}"))
        WV.append(wtile(wv_hd[l], [D, NH * HS], f"wv{l}"))
        WP.append((wtile(wpa[l], [128, D], f"wpA{l}"),
                   wtile(wpb[l], [64, D], f"wpB{l}")))
        WF1.append(wtile(wf1[l], [D, FF], f"wf1{l}"))
        WF2.append([wtile(wf2[l, 128 * f:128 * (f + 1), :], [128, D], f"wf2{l}_{f}")
                    for f in range(3)])

        def colv(src, tag, p=D):
            t_ = wp.tile([p, 1], F32, tag=tag, name=tag)
            nc.sync.dma_start(out=t_[:], in_=src[:, None])
            return t_
        L1G.append(colv(ln1_g[l], f"l1g{l}"))
        L1B.append(colv(ln1_b[l], f"l1b{l}"))
        L2G.append(colv(ln2_g[l], f"l2g{l}"))
        L2B.append(colv(ln2_b[l], f"l2b{l}"))
        BP.append(colv(b_proj[l], f"bp{l}"))
        BF2.append(colv(b_ff2[l], f"bf2{l}"))
        BF1.append([colv(b_ff1[l, 128 * f:128 * (f + 1)], f"bf1{l}_{f}", p=128)
                    for f in range(3)])

    lnfg = wp.tile([D, 1], F32, tag="lnfg")
    nc.sync.dma_start(out=lnfg[:], in_=lnf_g[:, None])
    lnfb = wp.tile([D, 1], F32, tag="lnfb")
    nc.sync.dma_start(out=lnfb[:], in_=lnf_b[:, None])
    wact = wtile(wact_d[:, :], [D, DA], "wact")

    # ---------------- embedding ----------------
    XT = []
    obsp_cm = tc.tile_pool(name="obsp", bufs=1)
    obsp = obsp_cm.__enter__()
    for b in range(BPC):
        with nc.named_scope(f"embed{b}"):
            obsT = obsp.tile([DO, T], F32, tag=f"obsT{b}", name=f"obsT{b}")
            for t in range(NT):
                otile = tp.tile([128, DO], F32, tag="otile")
                nc.sync.dma_start(out=otile[:], in_=obss[b, 128 * t:128 * (t + 1), :])
                ps = psum(DO, 128)
                nc.tensor.transpose(ps[:], otile[:], ident[:])
                nc.vector.tensor_copy(out=obsT[:, 128 * t:128 * (t + 1)], in_=ps[:])
            gT = tp.tile([DG, 1], F32, tag="gT")
            nc.sync.dma_start(out=gT[:], in_=goals[b, 0, :, None])
            cps = psum(D, 1)
            nc.tensor.matmul(cps[:], wgo_g[:], gT[:], start=True, stop=True)
            cgo = tp.tile([D, 1], F32, tag="cgo")
            nc.vector.tensor_add(out=cgo[:], in0=cps[:], in1=bgo[:])
            xT = xp.tile([D, T], F32, tag=f"xT{b}")
            for c in range(NCH):
                sl = slice(CW * c, CW * (c + 1))
                ps = psum(D, CW)
                nc.tensor.matmul(ps[:], wgo_o[:], obsT[:, sl],
                                 start=True, stop=True)
                nc.vector.tensor_scalar(out=xT[:, sl], in0=ps[:], scalar1=cgo[:],
                                        scalar2=None, op0=mybir.AluOpType.add)
                nc.vector.tensor_add(out=xT[:, sl], in0=xT[:, sl], in1=posT[:, sl])
            XT.append(xT)
    obsp_cm.__exit__(None, None, None)
    dbg = getattr(nc, "_dbg", None)
    if dbg:
        nc.sync.dma_start(out=dbg["xemb"][:, :], in_=XT[0][:])

    # ---------------- helpers ----------------
    def layernorm(xT, g, b, tag):
        """x [D, T] -> (x - mean)/sqrt(var+eps) * g + b, stats over D (partitions)."""
        xn = xnp.tile([D, T], BF16, tag=tag, name=tag)
        for c in range(NCH):
            sl = slice(CW * c, CW * (c + 1))
            x2c = tp.tile([D, CW], F32, tag="x2c")
            nc.vector.tensor_mul(out=x2c[:], in0=xT[:, sl], in1=xT[:, sl])
            ps1 = psum(1, CW)
            nc.tensor.matmul(ps1[:], ones96[:], xT[:, sl], start=True, stop=True)
            ps2 = psum(1, CW)
            nc.tensor.matmul(ps2[:], ones96[:], x2c[:], start=True, stop=True)
            # m = s1/96 ; var = s2/96 - m^2 ; rstd = exp(-0.5*ln(var+eps))
            m_sb = tp.tile([1, CW], F32, tag="m_sb")
            nc.vector.tensor_scalar(out=m_sb[:], in0=ps1[:], scalar1=1.0 / D,
                                    scalar2=None, op0=mybir.AluOpType.mult)
            msq = tp.tile([1, CW], F32, tag="msq")
            nc.vector.tensor_mul(out=msq[:], in0=m_sb[:], in1=m_sb[:])
            var = tp.tile([1, CW], F32, tag="var")
            nc.vector.tensor_scalar(out=var[:], in0=ps2[:], scalar1=1.0 / D,
                                    scalar2=None, op0=mybir.AluOpType.mult)
            nc.vector.tensor_sub(out=var[:], in0=var[:], in1=msq[:])
            nc.scalar.activation(out=var[:], in_=var[:],
                                 func=mybir.ActivationFunctionType.Ln,
                                 bias=eps_c[:])
            rs_sb = tp.tile([1, CW], F32, tag="rs_sb")
            nc.scalar.activation(out=rs_sb[:], in_=var[:],
                                 func=mybir.ActivationFunctionType.Exp, scale=-0.5)
            m_b = tp.tile([D, CW], F32, tag="m_b")
            rs_b = tp.tile([D, CW], F32, tag="rs_b")
            nc.gpsimd.partition_broadcast(m_b[:], m_sb[:])
            nc.gpsimd.partition_broadcast(rs_b[:], rs_sb[:])
            xnf32 = tp.tile([D, CW], F32, tag="xnf32")
            nc.vector.tensor_sub(out=xnf32[:], in0=xT[:, sl], in1=m_b[:])
            nc.vector.tensor_mul(out=xnf32[:], in0=xnf32[:], in1=rs_b[:])
            nc.vector.tensor_scalar(out=xn[:, sl], in0=xnf32[:], scalar1=g[:],
                                    scalar2=b[:], op0=mybir.AluOpType.mult,
                                    op1=mybir.AluOpType.add)
        return xn

    # ---------------- transformer layers ----------------
    for l in range(L):
        for b in range(BPC):
            xT = XT[b]
            with nc.named_scope(f"L{l}b{b}.ln1"):
                xn = layernorm(xT, L1G[l], L1B[l], "xn1")
            with nc.named_scope(f"L{l}b{b}.qkv"):
                # q/k spread tiles: head h<4 at rows 32h of A, h>=4 at 32(h-4) of B
                qA = qkp.tile([112, T], BF16, tag="qA")
                qB = qkp.tile([48, T], BF16, tag="qB")
                kA = qkp.tile([112, T], BF16, tag="kA")
                kB = qkp.tile([48, T], BF16, tag="kB")
                for c in range(NCH):
                    sl = slice(CW * c, CW * (c + 1))
                    for dst, w, scal in ((qA, WQA[l], SCALE), (qB, WQB[l], SCALE),
                                         (kA, WKA[l], None), (kB, WKB[l], None)):
                        p = dst.shape[0]
                        ps = psum(p, CW)
                        nc.tensor.matmul(ps[:], w[:], xn[:, sl], start=True, stop=True)
                        if scal is None:
                            nc.vector.tensor_copy(out=dst[:, sl], in_=ps[:])
                        else:
                            nc.vector.tensor_scalar(
                                out=dst[:, sl], in0=ps[:], scalar1=scal,
                                scalar2=None, op0=mybir.AluOpType.mult)
                # vT tiles: per token-tile t, head h at cols 96t+16h..+16
                vaug = vp.tile([128, 96 * NT], EDT, tag="vaug")
                for t in range(NT):
                    ps = psum(128, NH * HS)
                    nc.tensor.matmul(ps[:], xn[:, 128 * t:128 * (t + 1)], WV[l][:],
                                     start=True, stop=True)
                    nc.vector.tensor_copy(out=vaug[:, 96 * t:96 * (t + 1)], in_=ps[:])
            ocatA = op_.tile([128, T], BF16, tag="ocatA")
            ocatB = op_.tile([64, T], BF16, tag="ocatB")
            nc.vector.memset(ocatA[:], 0.0)
            nc.vector.memset(ocatB[:], 0.0)
            for c in range(NCH):
                smax = 4 * c + 3
                with nc.named_scope(f"L{l}b{b}.att{c}"):
                    oA = psum(128, CW, tag="ps")
                    oB = psum(128, CW, tag="ps")
                    dP = psum(128, CW, tag="ps")
                    dP2 = psum(64, CW, tag="ps")
                    nc.vector.memset(dP[:], 1.0)
                    nc.vector.memset(dP2[:], 1.0)
                    for s in range(smax + 1):
                        ets = []
                        for h in range(NH):
                            spr_k, spr_q, row = ((kA, qA, 32 * h) if h < 4
                                                 else (kB, qB, 32 * (h - 4)))
                            pqk = psum(128, CW)
                            nc.tensor.matmul(
                                pqk[:],
                                spr_k[row:row + HS, 128 * s:128 * (s + 1)],
                                spr_q[row:row + HS, CW * c:CW * (c + 1)],
                                start=True, stop=True, tile_position=(row, 0))
                            # NOTE: PE must not read an ACT-written tile
                            # directly (observed HW race); interpose DVE/GPSIMD.
                            et = ep.tile([128, CW], EDT, tag="et")
                            etx = ep.tile([128, CW], EDT, tag="etx")
                            nc.scalar.activation(
                                out=etx[:], in_=pqk[:],
                                func=mybir.ActivationFunctionType.Exp)
                            if s >= 4 * c:  # block crossed by the causal diagonal
                                rel = 128 * s - CW * c
                                nc.gpsimd.affine_select(
                                    out=et[:], in_=etx[:], pattern=[[1, CW]],
                                    compare_op=mybir.AluOpType.is_ge,
                                    fill=0.0, base=-rel, channel_multiplier=-1)
                            else:
                                nc.vector.tensor_copy(out=et[:], in_=etx[:])
                            ets.append(et)
                        for h in range(NH):
                            o, col = (oA, 32 * h) if h < 4 else (oB, 32 * (h - 4))
                            nc.tensor.matmul(
                                o[col:col + HS, :],
                                vaug[:, 96 * s + HS * h:96 * s + HS * (h + 1)],
                                ets[h][:],
                                start=(s == 0), stop=(s == smax),
                                tile_position=(0, col), skip_group_check=True)
                        for h in range(NH):
                            dn, dcol = (dP, 32 * h) if h < 4 else (dP2, 32 * (h - 4))
                            nc.tensor.matmul(
                                dn[dcol:dcol + HS, :], ones128[:], ets[h][:],
                                start=(s == 0), stop=(s == smax),
                                tile_position=(0, dcol), skip_group_check=True)
                    # reciprocal of (replicated) denominators in one base-0
                    # approx op per tile (custom DVE ops misbehave at
                    # non-zero base partitions), then per-head scale
                    rb = tp.tile([128, CW], F32, tag="rb")
                    rbB = tp.tile([64, CW], F32, tag="rbB")
                    nc.vector.reciprocal_approx_fast(out=rb[:], in_=dP[:])
                    nc.vector.reciprocal_approx_fast(out=rbB[:], in_=dP2[:])
                    for h in range(NH):
                        o, col = (oA, 32 * h) if h < 4 else (oB, 32 * (h - 4))
                        dcol = 32 * h if h < 4 else 32 * (h - 4)
                        oc = ocatA if h < 4 else ocatB
                        rt = rb if h < 4 else rbB
                        nc.vector.tensor_mul(
                            out=oc[dcol:dcol + HS, CW * c:CW * (c + 1)],
                            in0=o[col:col + HS, :], in1=rt[dcol:dcol + HS, :])
            if dbg and l == 0 and b == 0:
                dbgoc = tp.tile([128, T], F32, tag="dbgoc", name="dbgoc")
                nc.vector.tensor_copy(out=dbgoc[:], in_=ocatA[:])
                nc.sync.dma_start(out=dbg["oc0"][:, :], in_=dbgoc[:])
                dbgq = tp.tile([112, T], F32, tag="dbgq", name="dbgq")
                nc.vector.tensor_copy(out=dbgq[:], in_=qA[:])
                nc.sync.dma_start(out=dbg["qa0"][:, :], in_=dbgq[:])
                dbgv = tp.tile([128, 96 * NT], F32, tag="dbgv", name="dbgv")
                nc.vector.tensor_copy(out=dbgv[:], in_=vaug[:])
                nc.sync.dma_start(out=dbg["va0"][:, :], in_=dbgv[:])
            with nc.named_scope(f"L{l}b{b}.proj"):
                for c in range(NCH):
                    sl = slice(CW * c, CW * (c + 1))
                    ps = psum(D, CW)
                    nc.tensor.matmul(ps[:], WP[l][0][:], ocatA[:, sl],
                                     start=True, stop=False)
                    nc.tensor.matmul(ps[:], WP[l][1][:], ocatB[:, sl],
                                     start=False, stop=True)
                    nc.vector.tensor_scalar(out=ps[:], in0=ps[:], scalar1=BP[l][:],
                                            scalar2=None, op0=mybir.AluOpType.add)
                    nc.vector.tensor_add(out=xT[:, sl], in0=xT[:, sl], in1=ps[:])
            if dbg and l == 0 and b == 0:
                nc.sync.dma_start(out=dbg["xl0"][:, :], in_=xT[:])
            with nc.named_scope(f"L{l}b{b}.ln2"):
                xn2 = layernorm(xT, L2G[l], L2B[l], "xn2")
            with nc.named_scope(f"L{l}b{b}.ff"):
                h1 = [hp.tile([128, T], BF16, tag=f"h1_{f}", name=f"h1_{f}")
                      for f in range(3)]
                for c in range(NCH):
                    sl = slice(CW * c, CW * (c + 1))
                    for f in range(3):
                        ps = psum(128, CW)
                        nc.tensor.matmul(ps[:], WF1[l][:, 128 * f:128 * (f + 1)],
                                         xn2[:, sl], start=True, stop=True)
                        nc.vector.tensor_scalar(
                            out=h1[f][:, sl], in0=ps[:], scalar1=BF1[l][f][:],
                            scalar2=0.0, op0=mybir.AluOpType.add,
                            op1=mybir.AluOpType.max)
                for c in range(NCH):
                    sl = slice(CW * c, CW * (c + 1))
                    ps = psum(D, CW)
                    for f in range(3):
                        nc.tensor.matmul(ps[:], WF2[l][f][:], h1[f][:, sl],
                                         start=(f == 0), stop=(f == 2))
                    nc.vector.tensor_scalar(out=ps[:], in0=ps[:], scalar1=BF2[l][:],
                                            scalar2=None, op0=mybir.AluOpType.add)
                    nc.vector.tensor_add(out=xT[:, sl], in0=xT[:, sl], in1=ps[:])

    # ---------------- final LN + head ----------------
    for b in range(BPC):
        with nc.named_scope(f"head{b}"):
            xnf = layernorm(XT[b], lnfg, lnfb, "xn1")
            for t in range(NT):
                ps = psum(128, DA)
                nc.tensor.matmul(ps[:], xnf[:, 128 * t:128 * (t + 1)], wact[:],
                                 start=True, stop=True)
                osb = tp.tile([128, DA], F32, tag="osb")
                nc.vector.tensor_add(out=osb[:], in0=ps[:], in1=bact_b[:])
                nc.sync.dma_start(out=out[b, 128 * t:128 * (t + 1), :], in_=osb[:])

    ctx.close()


_CACHED = None


def _pin_act_tables():
    # Pin every ACT function to the one set containing both Ln and Exp so the
    # table is loaded once instead of thrashing between sets per activation.
    from concourse import hw_specs
    import concourse.bacc as bacc_mod
    if getattr(hw_specs, "_act_tables_pinned", False):
        return
    orig = hw_specs.get_activation_tables

    def pinned(arch):
        t = orig(arch)
        keep = "natural_log_exp_and_others"
        return {n: (f if n == keep else set()) for n, f in t.items()}

    hw_specs.get_activation_tables = pinned
    if hasattr(bacc_mod, "get_activation_tables"):
        bacc_mod.get_activation_tables = pinned
    hw_specs._act_tables_pinned = True


def _get_nc():
    global _CACHED
    if _CACHED is None:
        _pin_act_tables()
        nc = bacc.Bacc("TRN2", target_bir_lowering=False, debug=False,
                       enable_asserts=False)
        build(nc)
        nc.compile()
        _CACHED = nc
    return _CACHED


def prep_weights(inputs):
    """Host-side layout/casting of the (replicated) weights."""
    f = {k: _dt(v) for k, v in inputs.items()}
    o = {}
    o["w_go"] = f["w_go"]
    o["b_go"] = f["b_go"]
    o["posT"] = np.ascontiguousarray(f["pos_emb"].T)
    wq, wk, wv, wpj = f["wq"], f["wk"], f["wv"], f["w_proj"]
    qa = np.zeros((L, D, 112), np.float32)
    qb = np.zeros((L, D, 48), np.float32)
    ka = np.zeros((L, D, 112), np.float32)
    kb = np.zeros((L, D, 48), np.float32)
    pa = np.zeros((L, 128, D), np.float32)
    pb = np.zeros((L, 64, D), np.float32)
    for h in range(NH):
        if h < 4:
            qa[:, :, 32 * h:32 * h + HS] = wq[:, h]
            ka[:, :, 32 * h:32 * h + HS] = wk[:, h]
            pa[:, 32 * h:32 * h + HS, :] = wpj[:, HS * h:HS * (h + 1), :]
        else:
            r = 32 * (h - 4)
            qb[:, :, r:r + HS] = wq[:, h]
            kb[:, :, r:r + HS] = wk[:, h]
            pb[:, r:r + HS, :] = wpj[:, HS * h:HS * (h + 1), :]
    o["wqa"] = qa.astype(NPBF)
    o["wqb"] = qb.astype(NPBF)
    o["wka"] = ka.astype(NPBF)
    o["wkb"] = kb.astype(NPBF)
    o["wv_hd"] = np.ascontiguousarray(
        wv.transpose(0, 2, 1, 3).reshape(L, D, NH * HS)).astype(NPBF)
    o["wpa"] = pa.astype(NPBF)
    o["wpb"] = pb.astype(NPBF)
    o["wf1"] = f["w_ff1"].astype(NPBF)
    o["wf2"] = f["w_ff2"].astype(NPBF)
    o["wact"] = f["w_act"].astype(NPBF)
    for k in ("ln1_g", "ln1_b", "ln2_g", "ln2_b", "b_proj", "b_ff1", "b_ff2",
              "lnf_g", "lnf_b", "b_act"):
        o[k] = f[k]
    return o, f


def kernel(**inputs):
    global LAST_RESULT
    nc = _get_nc()
    w, f = prep_weights(inputs)
    in_maps = []
    for i in range(NCORES):
        m = dict(w)
        m["goals"] = f["goals"][BPC * i:BPC * (i + 1)]
        m["obss"] = f["obss"][BPC * i:BPC * (i + 1)]
        in_maps.append(m)
    res = run_bass_kernel_spmd(nc, in_maps, core_ids=list(range(NCORES)))
    LAST_RESULT = res
    return np.concatenate([r["out"] for r in res.results], axis=0)
